# revision 20
# baseline (speedup 1.0000x reference)
"""Single-head attention with per-sample padding masks on 8 Trainium2
NeuronCores.

kernel(**inputs) takes the FULL unsharded inputs (as produced by the
problem's setup_inputs) and returns the FULL [B, N, D] float32 output.

Two SPMD device programs (all cores always run the same instruction
stream; per-core differences are data only):

1. Banked (length-aware, used when the event_lengths fit the template):
   total real attention work is sum_b ceil(L_b/128)^2 tiles, which for
   skewed lengths is far less than B*16*16 dense tiles. The host packs
   (batch, query-tile-range) bins into a fixed per-core template:
     bank0: KV proj over J0=16 key-tiles + attention for G0=8 query-tiles
            (scores in fp8e4 DoubleRow: K=256 per pass, 2x matmul rate)
     bank1: KV proj over J1 (adaptive) key-tiles + attention for G1=2
            query-tiles, scores in bf16
     meanv-unit: meanv_b = mean(x_b) @ WvT + bv (for padded query rows)
   The V bias is folded into the output (A@(v+1*bv) = pav + rs*bv), and
   ~4us of warm-up matmuls on const tiles open the HAM clock gate while
   the input DMAs stream in.
   Each bin sees ALL valid keys of its batch (J >= ceil(L/128)), so no
   cross-core softmax stitching is needed. The host scatters bin outputs
   back to [B, N, D] and pastes meanv into padded rows.

2. Dense fallback (one batch per core) for length sets that don't fit.

Both use the S-transposed attention layout:
  ST = kT_tile.T @ qT_block   [128 j, w i]  (scores transposed: the exp
       output is directly the lhsT of the AV matmul -> no PE transposes)
  A  = exp(s*ST + maskj[jt])  maskj is a per-PARTITION bias (-1e9 for
       keys j >= L) applied by the activation
  pav = sum_jt A_jt.T @ v_jt  [128 i, 512 d]
  rs  = sum_jt A_jt.T @ ones  [128 i, 1]  (softmax denominator; shares
       the loaded weights with the pav matmul -> ~26 ns each)
  out = pav * (1/rs)
"""

import math
import sys
from contextlib import ExitStack

import numpy as np

sys.path.insert(0, "/opt/trn_rl_repo")

import ml_dtypes  # noqa: E402

import concourse.mybir as mybir  # noqa: E402
import concourse.tile as tile  # noqa: E402
from concourse import bacc  # noqa: E402

P = 128
B, N, D = 8, 2048, 512
FB = 512  # psum free-dim block (one bank)
MASK_VAL = -1.0e9
SCALE = 1.0 / math.sqrt(D)

# banked template (compile-time; uniform across cores). J1 adapts to the
# small-batch max at prepare() time; J0/G0/G1 are fixed.
J0, G0 = 16, 8
J1_MAX, G1 = 8, 2

f32 = mybir.dt.float32
Ident = mybir.ActivationFunctionType.Identity
Exp = mybir.ActivationFunctionType.Exp


def _blocks(total, blk=FB):
    s = 0
    while s < total:
        w = min(blk, total - s)
        yield s, w
        s += w


# --------------------------------------------------------------------------
# banked program
# --------------------------------------------------------------------------


def build_banked_nc(J1, n=N, d=D, mm_dt=mybir.dt.bfloat16, debug=False):
    dc_n = d // P
    Mult = mybir.AluOpType.mult
    Add = mybir.AluOpType.add

    nc = bacc.Bacc(None, target_bir_lowering=False, debug=debug)

    wqT_d = nc.declare_dram_parameter("wqT", [d, d], mm_dt, isOutput=False)
    wkT_d = nc.declare_dram_parameter("wkT", [d, d], mm_dt, isOutput=False)
    wvT_d = nc.declare_dram_parameter("wvT", [d, d], mm_dt, isOutput=False)
    bq_d = nc.declare_dram_parameter("bq", [d], f32, isOutput=False)
    bk_d = nc.declare_dram_parameter("bk", [d], f32, isOutput=False)
    bv_d = nc.declare_dram_parameter("bv1", [1, d], mm_dt, isOutput=False)
    xm_d = nc.declare_dram_parameter("xm", [d, n], mm_dt, isOutput=False)
    bank_d = []
    for bi, (J, G) in enumerate(((J0, G0), (J1, G1))):
        bank_d.append(
            {
                "xb": nc.declare_dram_parameter(
                    f"xb{bi}", [d, J * P], mm_dt, isOutput=False
                ),
                "xq": nc.declare_dram_parameter(
                    f"xq{bi}", [d, G * P], mm_dt, isOutput=False
                ),
                "maskj": nc.declare_dram_parameter(
                    f"maskj{bi}", [P, J], f32, isOutput=False
                ),
                "out": nc.declare_dram_parameter(
                    f"out{bi}", [G * P, d], f32, isOutput=True
                ),
            }
        )
    meanv_d = nc.declare_dram_parameter("meanv", [1, d], f32, isOutput=True)

    with tile.TileContext(nc) as tc, ExitStack() as ctx:
        const = ctx.enter_context(tc.tile_pool(name="const", bufs=1))
        big = ctx.enter_context(tc.tile_pool(name="big", bufs=1))
        work = ctx.enter_context(tc.tile_pool(name="work", bufs=2))
        small = ctx.enter_context(tc.tile_pool(name="small", bufs=4))
        psum_s = ctx.enter_context(tc.tile_pool(name="psum_s", bufs=2, space="PSUM"))
        psum_av = ctx.enter_context(tc.tile_pool(name="psum_av", bufs=2, space="PSUM"))
        psum_rs = ctx.enter_context(tc.tile_pool(name="psum_rs", bufs=2, space="PSUM"))
        psum_m = ctx.enter_context(tc.tile_pool(name="psum_m", bufs=1, space="PSUM"))

        ones1 = const.tile([1, P], mm_dt)
        nc.vector.memset(ones1, 1.0)
        onesc = const.tile([P, 1], mm_dt)
        nc.vector.memset(onesc, 1.0)
        bv_sb = const.tile([1, d], mm_dt)
        nc.sync.dma_start(out=bv_sb, in_=bv_d[:, :])
        bq_sb = const.tile([P, dc_n], f32)
        nc.sync.dma_start(out=bq_sb, in_=bq_d.ap().rearrange("(c p) -> p c", p=P))
        bk_sb = const.tile([P, dc_n], f32)
        nc.sync.dma_start(out=bk_sb, in_=bk_d.ap().rearrange("(c p) -> p c", p=P))

        wqT_sb = big.tile([P, dc_n, d], mm_dt)
        wkT_sb = big.tile([P, dc_n, d], mm_dt)
        wvT_sb = big.tile([P, dc_n, d], mm_dt)

        banks = []
        for bi, (J, G) in enumerate(((J0, G0), (J1, G1))):
            banks.append(
                {
                    "J": J,
                    "G": G,
                    "xb": big.tile([P, dc_n, J * P], mm_dt, name=f"xb{bi}_sb"),
                    "xq": big.tile([P, dc_n, G * P], mm_dt, name=f"xq{bi}_sb"),
                    "maskj": const.tile([P, J], f32, name=f"maskj{bi}_sb"),
                    "fp8": bi == 0,
                    "kT": big.tile(
                        [P, dc_n, J * P],
                        mybir.dt.float8e4 if bi == 0 else mm_dt,
                        name=f"kT{bi}_sb",
                    ),
                    "v": big.tile([P, J, d], mm_dt, name=f"v{bi}_sb"),
                    "qT": big.tile(
                        [P, dc_n, G * P],
                        mybir.dt.float8e4 if bi == 0 else mm_dt,
                        name=f"qT{bi}_sb",
                    ),
                    "d": bank_d[bi],
                }
            )
        xm_sb = big.tile([P, dc_n, n], mm_dt)

        # --- DMA in consumption order, FB-sized pieces so the first K-proj
        # groups never wait on whole slabs ---
        for dc in range(dc_n):
            nc.sync.dma_start(
                out=wkT_sb[:, dc, :], in_=wkT_d[dc * P : (dc + 1) * P, :]
            )
        xb0_d = banks[0]["d"]["xb"]
        for ib, w in _blocks(J0 * P):
            for dc in range(dc_n):
                nc.sync.dma_start(
                    out=banks[0]["xb"][:, dc, ib : ib + w],
                    in_=xb0_d[dc * P : (dc + 1) * P, ib : ib + w],
                )
            if ib == 0:
                for bk_ in banks:
                    nc.sync.dma_start(out=bk_["maskj"], in_=bk_["d"]["maskj"][:, :])
        for dc in range(dc_n):
            nc.sync.dma_start(
                out=wvT_sb[:, dc, :], in_=wvT_d[dc * P : (dc + 1) * P, :]
            )
        for dc in range(dc_n):
            nc.sync.dma_start(
                out=wqT_sb[:, dc, :], in_=wqT_d[dc * P : (dc + 1) * P, :]
            )
        xq0_d = banks[0]["d"]["xq"]
        for ib, w in _blocks(G0 * P):
            for dc in range(dc_n):
                nc.sync.dma_start(
                    out=banks[0]["xq"][:, dc, ib : ib + w],
                    in_=xq0_d[dc * P : (dc + 1) * P, ib : ib + w],
                )
        for dc in range(dc_n):
            nc.sync.dma_start(out=xm_sb[:, dc, :], in_=xm_d[dc * P : (dc + 1) * P, :])
        for dc in range(dc_n):
            nc.sync.dma_start(
                out=banks[1]["xb"][:, dc, :],
                in_=banks[1]["d"]["xb"][dc * P : (dc + 1) * P, :],
            )
            nc.sync.dma_start(
                out=banks[1]["xq"][:, dc, :],
                in_=banks[1]["d"]["xq"][dc * P : (dc + 1) * P, :],
            )

        # PE warm-up: ~8us of dummy matmuls (cold-clock rate) so the HAM
        # clock-gate opens and the PE stays busy through the whole input-DMA
        # ramp (fewer warm-up MMs re-throttles HAM and measures ~20us worse).
        zw = const.tile([P, P], mm_dt)
        nc.vector.memset(zw, 0.0)
        zr = const.tile([P, FB], mm_dt)
        nc.vector.memset(zr, 0.0)
        for _ in range(3):
            ps = psum_s.tile([P, FB], f32, tag="ps", name="warm_ps")
            for i in range(4):
                nc.tensor.matmul(ps, lhsT=zw, rhs=zr, start=(i == 0), stop=(i == 3))

        # bv broadcast to all 128 partitions once (K=1 matmul)
        pbv = psum_m.tile([P, d], f32, tag="pm")
        nc.tensor.matmul(pbv, lhsT=ones1, rhs=bv_sb, start=True, stop=True)
        bvb_sb = small.tile([P, d], f32)
        nc.vector.tensor_copy(bvb_sb, pbv)

        def proj_T(w_sb, b_sb, x_sb, o_sb, ncols, on_scalar=False):
            """o[e, i] = sum_d w[d, e] x[d, i] + b[e]  (qT/kT layout)."""
            for ib, w in _blocks(ncols):
                for ec in range(dc_n):
                    o_sl = o_sb[:, ec, ib : ib + w]
                    ps = psum_s.tile([P, FB], f32, tag="ps")
                    for dc in range(dc_n):
                        nc.tensor.matmul(
                            ps[:, :w],
                            lhsT=w_sb[:, dc, ec * P : (ec + 1) * P],
                            rhs=x_sb[:, dc, ib : ib + w],
                            start=(dc == 0),
                            stop=(dc == dc_n - 1),
                        )
                    if on_scalar:
                        nc.scalar.activation(
                            o_sl,
                            ps[:, :w],
                            Ident,
                            bias=b_sb[:, ec : ec + 1],
                            scale=1.0,
                        )
                    else:
                        nc.vector.tensor_scalar(
                            out=o_sl,
                            in0=ps[:, :w],
                            scalar1=b_sb[:, ec : ec + 1],
                            scalar2=None,
                            op0=Add,
                        )

        def run_bank(bk_, hook=None):
            J, G = bk_["J"], bk_["G"]
            xb, xq, v = bk_["xb"], bk_["xq"], bk_["v"]
            kT, qT, use_fp8 = bk_["kT"], bk_["qT"], bk_["fp8"]
            maskj = bk_["maskj"]
            out_d = bk_["d"]["out"]
            # K projection (bias on scalar engine: vector is the proj-phase
            # bottleneck otherwise)
            proj_T(wkT_sb, bk_sb, xb, kT, J * P, on_scalar=True)
            # V projection (v[j, d] layout, bias via vector add)
            for jt in range(J):
                ps = psum_av.tile([P, d], f32, tag="pav")
                for dc in range(dc_n):
                    nc.tensor.matmul(
                        ps,
                        lhsT=xb[:, dc, jt * P : (jt + 1) * P],
                        rhs=wvT_sb[:, dc, :],
                        start=(dc == 0),
                        stop=(dc == dc_n - 1),
                    )
                nc.scalar.activation(v[:, jt, :], ps, Ident, bias=0.0, scale=1.0)
            # Q projection
            proj_T(wqT_sb, bq_sb, xq, qT, G * P)
            # attention
            for ibl, w in _blocks(G * P):
                attnT = work.tile([P, J, FB], mm_dt, tag=f"attnT{J}")
                for jt in range(J):
                    ps = psum_s.tile([P, FB], f32, tag="ps")
                    if use_fp8:
                        for c in range(dc_n // 2):
                            nc.tensor.matmul(
                                ps[:, :w],
                                lhsT=kT[:, 2 * c : 2 * c + 2, jt * P : (jt + 1) * P],
                                rhs=qT[:, 2 * c : 2 * c + 2, ibl : ibl + w],
                                perf_mode=mybir.MatmulPerfMode.DoubleRow,
                                start=(c == 0),
                                stop=(c == dc_n // 2 - 1),
                            )
                    else:
                        for dc in range(dc_n):
                            nc.tensor.matmul(
                                ps[:, :w],
                                lhsT=kT[:, dc, jt * P : (jt + 1) * P],
                                rhs=qT[:, dc, ibl : ibl + w],
                                start=(dc == 0),
                                stop=(dc == dc_n - 1),
                            )
                    nc.scalar.activation(
                        attnT[:, jt, :w],
                        ps[:, :w],
                        Exp,
                        bias=maskj[:, jt : jt + 1],
                        scale=SCALE,
                    )
                for its in range(w // P):
                    pav = psum_av.tile([P, d], f32, tag="pav")
                    prs = psum_rs.tile([P, 1], f32, tag="prs")
                    for jt in range(J):
                        a_t = attnT[:, jt, its * P : (its + 1) * P]
                        nc.tensor.matmul(
                            pav,
                            lhsT=a_t,
                            rhs=v[:, jt, :],
                            start=(jt == 0),
                            stop=(jt == J - 1),
                        )
                        nc.tensor.matmul(
                            prs,
                            lhsT=a_t,
                            rhs=onesc,
                            start=(jt == 0),
                            stop=(jt == J - 1),
                        )
                    rinv = small.tile([P, 1], f32)
                    nc.vector.reciprocal(rinv, prs)
                    outt = work.tile([P, d], f32, tag="outt")
                    nc.vector.scalar_tensor_tensor(
                        out=outt,
                        in0=pav,
                        scalar=rinv,
                        in1=bvb_sb,
                        op0=Mult,
                        op1=Add,
                    )
                    r0 = ibl + its * P
                    nc.sync.dma_start(out=out_d[r0 : r0 + P, :], in_=outt)
                if hook is not None and ibl == 0:
                    hook()
                    hook = None

        def emit_meanv():
            # meanv = mean(x) @ WvT + bv; placed mid-attention so the
            # vector reduces and the 5 matmuls hide behind the S/AV stream.
            xmean_f = small.tile([P, dc_n], f32, name="xmean_f")
            for dc in range(dc_n):
                nc.vector.reduce_sum(
                    xmean_f[:, dc : dc + 1],
                    xm_sb[:, dc, :],
                    axis=mybir.AxisListType.X,
                )
            xmean_b = small.tile([P, dc_n], mm_dt, name="xmean_b")
            nc.vector.tensor_copy(xmean_b, xmean_f)
            pm = psum_m.tile([1, d], f32, tag="pm2", name="pm")
            for dc in range(dc_n):
                nc.tensor.matmul(
                    pm,
                    lhsT=xmean_b[:, dc : dc + 1],
                    rhs=wvT_sb[:, dc, :],
                    start=(dc == 0),
                    stop=(dc == dc_n - 1),
                )
            meanv_sb = small.tile([1, d], f32, name="meanv_sb")
            nc.scalar.activation(meanv_sb, pm, Ident, bias=0.0, scale=1.0 / n)
            meanv2_sb = small.tile([1, d], f32, name="meanv2_sb")
            nc.vector.scalar_tensor_tensor(
                out=meanv2_sb,
                in0=meanv_sb,
                scalar=1.0,
                in1=bvb_sb[0:1, :],
                op0=Mult,
                op1=Add,
            )
            nc.sync.dma_start(out=meanv_d[:, :], in_=meanv2_sb)

        run_bank(banks[0], hook=emit_meanv)
        run_bank(banks[1])

    nc.compile()
    return nc


def make_banked_schedule(lens):
    """Pack (batch, query-tile-start) bins into the template.

    Returns (bank0_bins, bank1_bins, J1) with bins padded to length B
    with None, or None if this length multiset doesn't fit.
    """
    nti = [(int(L) + P - 1) // P for L in lens]
    bank0, bank1 = [], []
    small_ntj = [t for t in nti if t <= J1_MAX]
    J1 = max(small_ntj) if small_ntj else 1
    for b, L in enumerate(lens):
        ntj = nti[b]
        if ntj > J1:
            if ntj > J0:
                return None
            for s in range(0, nti[b], G0):
                bank0.append((b, s))
        else:
            for s in range(0, nti[b], G1):
                bank1.append((b, s))
    if len(bank0) > B or len(bank1) > B:
        return None
    bank0 += [None] * (B - len(bank0))
    bank1 += [None] * (B - len(bank1))
    return bank0, bank1, J1


def make_banked_in_maps(x, lens, wqT, wkT, wvT, bq, bk, bv1, sched):
    bf16 = ml_dtypes.bfloat16
    bank0, bank1, J1 = sched
    xT = [np.ascontiguousarray(x[b].T).astype(bf16) for b in range(B)]
    idx = np.arange(N)

    def maskj_for(L, J):
        m = np.where(idx[: J * P] < L, 0.0, MASK_VAL).astype(np.float32)
        return np.ascontiguousarray(m.reshape(J, P).T)  # [P, J]

    zmask0 = np.zeros((P, J0), np.float32)
    zmask1 = np.zeros((P, J1), np.float32)
    zx = np.zeros((D, 1), bf16)  # broadcast-compatible zero source

    in_maps = []
    for c in range(B):
        im = {
            "wqT": wqT,
            "wkT": wkT,
            "wvT": wvT,
            "bq": bq,
            "bk": bk,
            "bv1": bv1,
            "xm": xT[c],
        }
        for bi, (J, G, bins, zmask) in enumerate(
            ((J0, G0, bank0, zmask0), (J1, G1, bank1, zmask1))
        ):
            bin_ = bins[c]
            if bin_ is None:
                im[f"xb{bi}"] = np.zeros((D, J * P), bf16)
                im[f"xq{bi}"] = np.zeros((D, G * P), bf16)
                im[f"maskj{bi}"] = zmask
            else:
                b, s = bin_
                im[f"xb{bi}"] = np.ascontiguousarray(xT[b][:, : J * P])
                im[f"xq{bi}"] = np.ascontiguousarray(
                    xT[b][:, s * P : (s + G) * P]
                )
                im[f"maskj{bi}"] = maskj_for(int(lens[b]), J)
        in_maps.append(im)
    return in_maps


def assemble_banked(results, lens, sched):
    bank0, bank1, _J1 = sched
    out = np.empty((B, N, D), np.float32)
    meanv = {}
    for c in range(B):
        meanv[c] = np.asarray(results[c]["meanv"], np.float32).reshape(D)
    for b in range(B):
        out[b, :, :] = meanv[b][None, :]
    for bi, (G, bins) in enumerate(((G0, bank0), (G1, bank1))):
        for c in range(B):
            bin_ = bins[c]
            if bin_ is None:
                continue
            b, s = bin_
            L = int(lens[b])
            r0 = s * P
            r1 = min((s + G) * P, L)
            if r1 <= r0:
                continue
            o = np.asarray(results[c][f"out{bi}"], np.float32)
            out[b, r0:r1, :] = o[: r1 - r0, :]
    return out


# --------------------------------------------------------------------------
# dense fallback program (one batch per core)
# --------------------------------------------------------------------------


def build_attention_nc(n=N, d=D, mm_dt=mybir.dt.bfloat16, debug=False):
    """Dense per-batch program (fallback). S-transposed layout."""
    dc_n = d // P
    nt = n // P
    nb = n // FB
    Mult = mybir.AluOpType.mult
    Add = mybir.AluOpType.add

    nc = bacc.Bacc(None, target_bir_lowering=False, debug=debug)

    xT_d = nc.declare_dram_parameter("xT", [d, n], mm_dt, isOutput=False)
    wqT_d = nc.declare_dram_parameter("wqT", [d, d], mm_dt, isOutput=False)
    wkT_d = nc.declare_dram_parameter("wkT", [d, d], mm_dt, isOutput=False)
    wvT_d = nc.declare_dram_parameter("wvT", [d, d], mm_dt, isOutput=False)
    bq_d = nc.declare_dram_parameter("bq", [d], f32, isOutput=False)
    bk_d = nc.declare_dram_parameter("bk", [d], f32, isOutput=False)
    bv_d = nc.declare_dram_parameter("bv1", [1, d], mm_dt, isOutput=False)
    maskj_d = nc.declare_dram_parameter("maskj", [P, nt], f32, isOutput=False)
    out_d = nc.declare_dram_parameter("out", [n, d], f32, isOutput=True)
    meanv_d = nc.declare_dram_parameter("meanv", [1, d], f32, isOutput=True)

    with tile.TileContext(nc) as tc, ExitStack() as ctx:
        const = ctx.enter_context(tc.tile_pool(name="const", bufs=1))
        big = ctx.enter_context(tc.tile_pool(name="big", bufs=1))
        work = ctx.enter_context(tc.tile_pool(name="work", bufs=2))
        small = ctx.enter_context(tc.tile_pool(name="small", bufs=4))
        psum_s = ctx.enter_context(tc.tile_pool(name="psum_s", bufs=2, space="PSUM"))
        psum_av = ctx.enter_context(tc.tile_pool(name="psum_av", bufs=2, space="PSUM"))
        psum_rs = ctx.enter_context(tc.tile_pool(name="psum_rs", bufs=2, space="PSUM"))
        psum_m = ctx.enter_context(tc.tile_pool(name="psum_m", bufs=1, space="PSUM"))

        ones1 = const.tile([1, P], mm_dt)
        nc.vector.memset(ones1, 1.0)
        onesc = const.tile([P, 1], mm_dt)
        nc.vector.memset(onesc, 1.0)
        maskj_sb = const.tile([P, nt], f32)
        nc.sync.dma_start(out=maskj_sb, in_=maskj_d[:, :])
        bv_sb = const.tile([1, d], mm_dt)
        nc.sync.dma_start(out=bv_sb, in_=bv_d[:, :])
        bq_sb = const.tile([P, dc_n], f32)
        nc.sync.dma_start(out=bq_sb, in_=bq_d.ap().rearrange("(c p) -> p c", p=P))
        bk_sb = const.tile([P, dc_n], f32)
        nc.sync.dma_start(out=bk_sb, in_=bk_d.ap().rearrange("(c p) -> p c", p=P))

        xT_sb = big.tile([P, dc_n, n], mm_dt)
        wqT_sb = big.tile([P, dc_n, d], mm_dt)
        wkT_sb = big.tile([P, dc_n, d], mm_dt)
        wvT_sb = big.tile([P, dc_n, d], mm_dt)
        for dc in range(dc_n):
            nc.sync.dma_start(
                out=wqT_sb[:, dc, :], in_=wqT_d[dc * P : (dc + 1) * P, :]
            )
        for dc in range(dc_n):
            nc.sync.dma_start(
                out=xT_sb[:, dc, :FB], in_=xT_d[dc * P : (dc + 1) * P, :FB]
            )
        for dc in range(dc_n):
            nc.sync.dma_start(
                out=wkT_sb[:, dc, :], in_=wkT_d[dc * P : (dc + 1) * P, :]
            )
        for dc in range(dc_n):
            nc.sync.dma_start(
                out=xT_sb[:, dc, FB:], in_=xT_d[dc * P : (dc + 1) * P, FB:]
            )
        for dc in range(dc_n):
            nc.sync.dma_start(
                out=wvT_sb[:, dc, :], in_=wvT_d[dc * P : (dc + 1) * P, :]
            )

        qT_sb = big.tile([P, dc_n, n], mm_dt)
        kT_sb = big.tile([P, dc_n, n], mm_dt)
        v_sb = big.tile([P, nt, d], mm_dt)

        for wT_sb, b_sb, oT_sb in ((wqT_sb, bq_sb, qT_sb), (wkT_sb, bk_sb, kT_sb)):
            for ib in range(nb):
                for ec in range(dc_n):
                    ps = psum_s.tile([P, FB], f32, tag="ps")
                    for dc in range(dc_n):
                        nc.tensor.matmul(
                            ps,
                            lhsT=wT_sb[:, dc, ec * P : (ec + 1) * P],
                            rhs=xT_sb[:, dc, ib * FB : (ib + 1) * FB],
                            start=(dc == 0),
                            stop=(dc == dc_n - 1),
                        )
                    nc.vector.tensor_scalar(
                        out=oT_sb[:, ec, ib * FB : (ib + 1) * FB],
                        in0=ps,
                        scalar1=b_sb[:, ec : ec + 1],
                        scalar2=None,
                        op0=Add,
                    )

        pbv = psum_m.tile([P, d], f32, tag="pm")
        nc.tensor.matmul(pbv, lhsT=ones1, rhs=bv_sb, start=True, stop=True)
        bvb_sb = small.tile([P, d], f32)
        nc.vector.tensor_copy(bvb_sb, pbv)

        for jt in range(nt):
            ps = psum_av.tile([P, d], f32, tag="pav")
            for dc in range(dc_n):
                nc.tensor.matmul(
                    ps,
                    lhsT=xT_sb[:, dc, jt * P : (jt + 1) * P],
                    rhs=wvT_sb[:, dc, :],
                    start=(dc == 0),
                    stop=(dc == dc_n - 1),
                )
            nc.vector.scalar_tensor_tensor(
                out=v_sb[:, jt, :], in0=ps, scalar=1.0, in1=bvb_sb, op0=Mult, op1=Add
            )

        xmean_f = small.tile([P, dc_n], f32)
        for dc in range(dc_n):
            nc.vector.reduce_sum(
                xmean_f[:, dc : dc + 1], xT_sb[:, dc, :], axis=mybir.AxisListType.X
            )
        xmean_b = small.tile([P, dc_n], mm_dt)
        nc.vector.tensor_copy(xmean_b, xmean_f)
        pm = psum_m.tile([1, d], f32, tag="pm2")
        for dc in range(dc_n):
            nc.tensor.matmul(
                pm,
                lhsT=xmean_b[:, dc : dc + 1],
                rhs=wvT_sb[:, dc, :],
                start=(dc == 0),
                stop=(dc == dc_n - 1),
            )
        meanv_sb = small.tile([1, d], f32)
        nc.scalar.activation(meanv_sb, pm, Ident, bias=0.0, scale=1.0 / n)
        meanv2_sb = small.tile([1, d], f32)
        nc.vector.scalar_tensor_tensor(
            out=meanv2_sb,
            in0=meanv_sb,
            scalar=1.0,
            in1=bvb_sb[0:1, :],
            op0=Mult,
            op1=Add,
        )
        nc.sync.dma_start(out=meanv_d[:, :], in_=meanv2_sb)

        for ibl in range(nb):
            attnT = work.tile([P, nt, FB], mm_dt)
            for jt in range(nt):
                ps = psum_s.tile([P, FB], f32, tag="ps")
                for dc in range(dc_n):
                    nc.tensor.matmul(
                        ps,
                        lhsT=kT_sb[:, dc, jt * P : (jt + 1) * P],
                        rhs=qT_sb[:, dc, ibl * FB : (ibl + 1) * FB],
                        start=(dc == 0),
                        stop=(dc == dc_n - 1),
                    )
                nc.scalar.activation(
                    attnT[:, jt, :],
                    ps,
                    Exp,
                    bias=maskj_sb[:, jt : jt + 1],
                    scale=SCALE,
                )
            for its in range(nb):
                it = ibl * nb + its
                pav = psum_av.tile([P, d], f32, tag="pav")
                prs = psum_rs.tile([P, 1], f32, tag="prs")
                for jt in range(nt):
                    a_t = attnT[:, jt, its * P : (its + 1) * P]
                    nc.tensor.matmul(
                        pav,
                        lhsT=a_t,
                        rhs=v_sb[:, jt, :],
                        start=(jt == 0),
                        stop=(jt == nt - 1),
                    )
                    nc.tensor.matmul(
                        prs,
                        lhsT=a_t,
                        rhs=onesc,
                        start=(jt == 0),
                        stop=(jt == nt - 1),
                    )
                rinv = small.tile([P, 1], f32)
                nc.vector.reciprocal(rinv, prs)
                outt = work.tile([P, d], f32, tag="outt")
                nc.vector.tensor_scalar(
                    out=outt, in0=pav, scalar1=rinv, scalar2=None, op0=Mult
                )
                nc.sync.dma_start(out=out_d[it * P : (it + 1) * P, :], in_=outt)

    nc.compile()
    return nc


def _marshal_weights(Wq, bq, Wk, bk, Wv, bv):
    bf16 = ml_dtypes.bfloat16
    return (
        np.ascontiguousarray(np.asarray(Wq, np.float32).T).astype(bf16),
        np.ascontiguousarray(np.asarray(Wk, np.float32).T).astype(bf16),
        np.ascontiguousarray(np.asarray(Wv, np.float32).T).astype(bf16),
        np.asarray(bq, np.float32),
        np.asarray(bk, np.float32),
        np.asarray(bv, np.float32).reshape(1, D).astype(bf16),
    )


def make_in_maps(x, event_lengths, Wq, bq, Wk, bk, Wv, bv, n=N, d=D):
    """Dense fallback marshaling: one batch element per core."""
    bf16 = ml_dtypes.bfloat16
    x = np.asarray(x, dtype=np.float32)
    lens = np.asarray(event_lengths).astype(np.int64)
    wqT, wkT, wvT, bq, bk, bv1 = _marshal_weights(Wq, bq, Wk, bk, Wv, bv)
    idx = np.arange(n)
    in_maps = []
    for b in range(x.shape[0]):
        L = int(lens[b])
        maskj = np.where(idx < L, 0.0, MASK_VAL).astype(np.float32)
        maskj = np.ascontiguousarray(maskj.reshape(n // P, P).T)
        in_maps.append(
            {
                "xT": np.ascontiguousarray(x[b].T).astype(bf16),
                "wqT": wqT,
                "wkT": wkT,
                "wvT": wvT,
                "bq": bq,
                "bk": bk,
                "bv1": bv1,
                "maskj": maskj,
            }
        )
    return in_maps


def assemble_output(results, event_lengths, n=N, d=D):
    """Dense fallback: stack per-core outputs; paste meanv into padded rows."""
    lens = np.asarray(event_lengths).astype(np.int64)
    outs = []
    for b, r in enumerate(results):
        o = np.array(r["out"], np.float32)
        L = int(lens[b])
        if L < n:
            o[L:, :] = np.asarray(r["meanv"], np.float32).reshape(1, d)
        outs.append(o)
    return np.stack(outs, axis=0)


_NC_CACHE = {}


def prepare(x, event_lengths, Wq, bq, Wk, bk, Wv, bv):
    """Returns (nc, in_maps, assemble_fn)."""
    x = np.asarray(x, dtype=np.float32)
    lens = np.asarray(event_lengths).astype(np.int64)
    sched = make_banked_schedule(lens)
    if sched is not None:
        key = ("banked", sched[2])
        if key not in _NC_CACHE:
            _NC_CACHE[key] = build_banked_nc(sched[2])
        wqT, wkT, wvT, bqf, bkf, bv1 = _marshal_weights(Wq, bq, Wk, bk, Wv, bv)
        in_maps = make_banked_in_maps(x, lens, wqT, wkT, wvT, bqf, bkf, bv1, sched)
        return (
            _NC_CACHE[key],
            in_maps,
            lambda results: assemble_banked(results, lens, sched),
        )
    if "dense" not in _NC_CACHE:
        _NC_CACHE["dense"] = build_attention_nc()
    in_maps = make_in_maps(x, event_lengths, Wq, bq, Wk, bk, Wv, bv)
    return (
        _NC_CACHE["dense"],
        in_maps,
        lambda results: assemble_output(results, event_lengths),
    )


def kernel(x, event_lengths, Wq, bq, Wk, bk, Wv, bv):
    from concourse.bass_utils import run_bass_kernel_spmd

    nc, in_maps, assemble = prepare(x, event_lengths, Wq, bq, Wk, bk, Wv, bv)
    res = run_bass_kernel_spmd(nc, in_maps, core_ids=list(range(B)))
    return assemble(res.results)


# revision 21
# speedup vs baseline: 1.1773x; 1.1773x over previous
"""Single-head attention with per-sample padding masks on 8 Trainium2
NeuronCores.

kernel(**inputs) takes the FULL unsharded inputs (as produced by the
problem's setup_inputs) and returns the FULL [B, N, D] float32 output.

Two SPMD device programs (all cores always run the same instruction
stream; per-core differences are data only):

1. Banked (length-aware, used when the event_lengths fit the template):
   total real attention work is sum_b ceil(L_b/128)^2 tiles, which for
   skewed lengths is far less than B*16*16 dense tiles. The host packs
   (batch, query-tile-range) bins into a fixed per-core template:
     bank0: KV proj over J0=16 key-tiles + attention for G0=8 query-tiles
            (scores in fp8e4 DoubleRow: K=256 per pass, 2x matmul rate)
     bank1: KV proj over J1 (adaptive) key-tiles + attention for G1=2
            query-tiles, scores in bf16
     meanv-unit: meanv_b = mean(x_b) @ WvT + bv (for padded query rows)
   The V bias is folded into the output (A@(v+1*bv) = pav + rs*bv), and
   ~4us of warm-up matmuls on const tiles open the HAM clock gate while
   the input DMAs stream in.
   Each bin sees ALL valid keys of its batch (J >= ceil(L/128)), so no
   cross-core softmax stitching is needed. The host scatters bin outputs
   back to [B, N, D] and pastes meanv into padded rows.

2. Dense fallback (one batch per core) for length sets that don't fit.

Both use the S-transposed attention layout:
  ST = kT_tile.T @ qT_block   [128 j, w i]  (scores transposed: the exp
       output is directly the lhsT of the AV matmul -> no PE transposes)
  A  = exp(s*ST + maskj[jt])  maskj is a per-PARTITION bias (-1e9 for
       keys j >= L) applied by the activation
  pav = sum_jt A_jt.T @ v_jt  [128 i, 512 d]
  rs  = sum_jt A_jt.T @ ones  [128 i, 1]  (softmax denominator; shares
       the loaded weights with the pav matmul -> ~26 ns each)
  out = pav * (1/rs)
"""

import math
import sys
from contextlib import ExitStack

import numpy as np

sys.path.insert(0, "/opt/trn_rl_repo")

import ml_dtypes  # noqa: E402

import concourse.mybir as mybir  # noqa: E402
import concourse.tile as tile  # noqa: E402
from concourse import bacc  # noqa: E402

P = 128
B, N, D = 8, 2048, 512
FB = 512  # psum free-dim block (one bank)
MASK_VAL = -1.0e9
SCALE = 1.0 / math.sqrt(D)

# banked template (compile-time; uniform across cores). J1 adapts to the
# small-batch max at prepare() time; J0/G0/G1 are fixed.
J0, G0 = 16, 8
J1_MAX, G1 = 8, 2

f32 = mybir.dt.float32
Ident = mybir.ActivationFunctionType.Identity
Exp = mybir.ActivationFunctionType.Exp


def _blocks(total, blk=FB):
    s = 0
    while s < total:
        w = min(blk, total - s)
        yield s, w
        s += w


# --------------------------------------------------------------------------
# banked program
# --------------------------------------------------------------------------


def build_banked_nc(J1, n=N, d=D, mm_dt=mybir.dt.bfloat16, debug=False):
    dc_n = d // P
    Mult = mybir.AluOpType.mult
    Add = mybir.AluOpType.add

    nc = bacc.Bacc(None, target_bir_lowering=False, debug=debug)

    wqT_d = nc.declare_dram_parameter("wqT", [d, d], mm_dt, isOutput=False)
    wkT_d = nc.declare_dram_parameter("wkT", [d, d], mm_dt, isOutput=False)
    wvT_d = nc.declare_dram_parameter("wvT", [d, d], mm_dt, isOutput=False)
    bq_d = nc.declare_dram_parameter("bq", [d], f32, isOutput=False)
    bk_d = nc.declare_dram_parameter("bk", [d], f32, isOutput=False)
    bv_d = nc.declare_dram_parameter("bv1", [1, d], mm_dt, isOutput=False)
    xm_d = nc.declare_dram_parameter("xm", [d, n], mm_dt, isOutput=False)
    bank_d = []
    for bi, (J, G) in enumerate(((J0, G0), (J1, G1))):
        bank_d.append(
            {
                "xb": nc.declare_dram_parameter(
                    f"xb{bi}", [d, J * P], mm_dt, isOutput=False
                ),
                "xq": nc.declare_dram_parameter(
                    f"xq{bi}", [d, G * P], mm_dt, isOutput=False
                ),
                "maskj": nc.declare_dram_parameter(
                    f"maskj{bi}", [P, J], f32, isOutput=False
                ),
                "out": nc.declare_dram_parameter(
                    f"out{bi}", [G * P, d], f32, isOutput=True
                ),
            }
        )
    meanv_d = nc.declare_dram_parameter("meanv", [1, d], f32, isOutput=True)

    with tile.TileContext(nc) as tc, ExitStack() as ctx:
        const = ctx.enter_context(tc.tile_pool(name="const", bufs=1))
        big = ctx.enter_context(tc.tile_pool(name="big", bufs=1))
        work = ctx.enter_context(tc.tile_pool(name="work", bufs=2))
        small = ctx.enter_context(tc.tile_pool(name="small", bufs=4))
        psum_s = ctx.enter_context(tc.tile_pool(name="psum_s", bufs=2, space="PSUM"))
        psum_av = ctx.enter_context(tc.tile_pool(name="psum_av", bufs=2, space="PSUM"))
        psum_rs = ctx.enter_context(tc.tile_pool(name="psum_rs", bufs=2, space="PSUM"))
        psum_m = ctx.enter_context(tc.tile_pool(name="psum_m", bufs=1, space="PSUM"))

        ones1 = const.tile([1, P], mm_dt)
        nc.vector.memset(ones1, 1.0)
        onesc = const.tile([P, 1], mm_dt)
        nc.vector.memset(onesc, 1.0)
        bv_sb = const.tile([1, d], mm_dt)
        nc.sync.dma_start(out=bv_sb, in_=bv_d[:, :])
        bq_sb = const.tile([P, dc_n], f32)
        nc.sync.dma_start(out=bq_sb, in_=bq_d.ap().rearrange("(c p) -> p c", p=P))
        bk_sb = const.tile([P, dc_n], f32)
        nc.sync.dma_start(out=bk_sb, in_=bk_d.ap().rearrange("(c p) -> p c", p=P))

        wqT_sb = big.tile([P, dc_n, d], mm_dt)
        wkT_sb = big.tile([P, dc_n, d], mm_dt)
        wvT_sb = big.tile([P, dc_n, d], mm_dt)

        banks = []
        for bi, (J, G) in enumerate(((J0, G0), (J1, G1))):
            banks.append(
                {
                    "J": J,
                    "G": G,
                    "xb": big.tile([P, dc_n, J * P], mm_dt, name=f"xb{bi}_sb"),
                    "xq": big.tile([P, dc_n, G * P], mm_dt, name=f"xq{bi}_sb"),
                    "maskj": const.tile([P, J], f32, name=f"maskj{bi}_sb"),
                    "fp8": bi == 0,
                    "kT": big.tile(
                        [P, dc_n, J * P],
                        mybir.dt.float8e4 if bi == 0 else mm_dt,
                        name=f"kT{bi}_sb",
                    ),
                    "v": big.tile([P, J, d], mm_dt, name=f"v{bi}_sb"),
                    "qT": big.tile(
                        [P, dc_n, G * P],
                        mybir.dt.float8e4 if bi == 0 else mm_dt,
                        name=f"qT{bi}_sb",
                    ),
                    "d": bank_d[bi],
                }
            )
        xm_sb = big.tile([P, dc_n, n], mm_dt)

        # --- DMA in consumption order, FB-sized pieces so the first K-proj
        # groups never wait on whole slabs ---
        for dc in range(dc_n):
            nc.sync.dma_start(
                out=wkT_sb[:, dc, :], in_=wkT_d[dc * P : (dc + 1) * P, :]
            )
        xb0_d = banks[0]["d"]["xb"]
        for ib, w in _blocks(J0 * P):
            for dc in range(dc_n):
                nc.sync.dma_start(
                    out=banks[0]["xb"][:, dc, ib : ib + w],
                    in_=xb0_d[dc * P : (dc + 1) * P, ib : ib + w],
                )
            if ib == 0:
                for bk_ in banks:
                    nc.sync.dma_start(out=bk_["maskj"], in_=bk_["d"]["maskj"][:, :])
        for dc in range(dc_n):
            nc.sync.dma_start(
                out=wvT_sb[:, dc, :], in_=wvT_d[dc * P : (dc + 1) * P, :]
            )
        for dc in range(dc_n):
            nc.sync.dma_start(
                out=wqT_sb[:, dc, :], in_=wqT_d[dc * P : (dc + 1) * P, :]
            )
        xq0_d = banks[0]["d"]["xq"]
        for ib, w in _blocks(G0 * P):
            for dc in range(dc_n):
                nc.sync.dma_start(
                    out=banks[0]["xq"][:, dc, ib : ib + w],
                    in_=xq0_d[dc * P : (dc + 1) * P, ib : ib + w],
                )
        for dc in range(dc_n):
            nc.sync.dma_start(out=xm_sb[:, dc, :], in_=xm_d[dc * P : (dc + 1) * P, :])
        for dc in range(dc_n):
            nc.sync.dma_start(
                out=banks[1]["xb"][:, dc, :],
                in_=banks[1]["d"]["xb"][dc * P : (dc + 1) * P, :],
            )
            nc.sync.dma_start(
                out=banks[1]["xq"][:, dc, :],
                in_=banks[1]["d"]["xq"][dc * P : (dc + 1) * P, :],
            )

        # PE warm-up: ~8us of dummy matmuls (cold-clock rate) so the HAM
        # clock-gate opens and the PE stays busy through the whole input-DMA
        # ramp (fewer warm-up MMs re-throttles HAM and measures ~20us worse).
        zw = const.tile([P, P], mm_dt)
        nc.vector.memset(zw, 0.0)
        zr = const.tile([P, FB], mm_dt)
        nc.vector.memset(zr, 0.0)
        for _ in range(5):
            ps = psum_s.tile([P, FB], f32, tag="ps", name="warm_ps")
            for i in range(4):
                nc.tensor.matmul(ps, lhsT=zw, rhs=zr, start=(i == 0), stop=(i == 3))

        # bv broadcast to all 128 partitions once (K=1 matmul)
        pbv = psum_m.tile([P, d], f32, tag="pm")
        nc.tensor.matmul(pbv, lhsT=ones1, rhs=bv_sb, start=True, stop=True)
        bvb_sb = small.tile([P, d], f32)
        nc.vector.tensor_copy(bvb_sb, pbv)

        def proj_T(w_sb, b_sb, x_sb, o_sb, ncols, on_scalar=False):
            """o[e, i] = sum_d w[d, e] x[d, i] + b[e]  (qT/kT layout)."""
            for ib, w in _blocks(ncols):
                for ec in range(dc_n):
                    o_sl = o_sb[:, ec, ib : ib + w]
                    ps = psum_s.tile([P, FB], f32, tag="ps")
                    for dc in range(dc_n):
                        nc.tensor.matmul(
                            ps[:, :w],
                            lhsT=w_sb[:, dc, ec * P : (ec + 1) * P],
                            rhs=x_sb[:, dc, ib : ib + w],
                            start=(dc == 0),
                            stop=(dc == dc_n - 1),
                        )
                    if on_scalar:
                        nc.scalar.activation(
                            o_sl,
                            ps[:, :w],
                            Ident,
                            bias=b_sb[:, ec : ec + 1],
                            scale=1.0,
                        )
                    else:
                        nc.vector.tensor_scalar(
                            out=o_sl,
                            in0=ps[:, :w],
                            scalar1=b_sb[:, ec : ec + 1],
                            scalar2=None,
                            op0=Add,
                        )

        def run_bank(bk_, hook=None):
            J, G = bk_["J"], bk_["G"]
            xb, xq, v = bk_["xb"], bk_["xq"], bk_["v"]
            kT, qT, use_fp8 = bk_["kT"], bk_["qT"], bk_["fp8"]
            maskj = bk_["maskj"]
            out_d = bk_["d"]["out"]
            # K projection (bias on scalar engine: vector is the proj-phase
            # bottleneck otherwise)
            proj_T(wkT_sb, bk_sb, xb, kT, J * P, on_scalar=True)
            # V projection (v[j, d] layout, bias via vector add)
            for jt in range(J):
                ps = psum_av.tile([P, d], f32, tag="pav")
                for dc in range(dc_n):
                    nc.tensor.matmul(
                        ps,
                        lhsT=xb[:, dc, jt * P : (jt + 1) * P],
                        rhs=wvT_sb[:, dc, :],
                        start=(dc == 0),
                        stop=(dc == dc_n - 1),
                    )
                nc.scalar.activation(v[:, jt, :], ps, Ident, bias=0.0, scale=1.0)
            # Q projection
            proj_T(wqT_sb, bq_sb, xq, qT, G * P)
            # attention
            for ibl, w in _blocks(G * P):
                attnT = work.tile([P, J, FB], mm_dt, tag=f"attnT{J}")
                for jt in range(J):
                    ps = psum_s.tile([P, FB], f32, tag="ps")
                    if use_fp8:
                        for c in range(dc_n // 2):
                            nc.tensor.matmul(
                                ps[:, :w],
                                lhsT=kT[:, 2 * c : 2 * c + 2, jt * P : (jt + 1) * P],
                                rhs=qT[:, 2 * c : 2 * c + 2, ibl : ibl + w],
                                perf_mode=mybir.MatmulPerfMode.DoubleRow,
                                start=(c == 0),
                                stop=(c == dc_n // 2 - 1),
                            )
                    else:
                        for dc in range(dc_n):
                            nc.tensor.matmul(
                                ps[:, :w],
                                lhsT=kT[:, dc, jt * P : (jt + 1) * P],
                                rhs=qT[:, dc, ibl : ibl + w],
                                start=(dc == 0),
                                stop=(dc == dc_n - 1),
                            )
                    nc.scalar.activation(
                        attnT[:, jt, :w],
                        ps[:, :w],
                        Exp,
                        bias=maskj[:, jt : jt + 1],
                        scale=SCALE,
                    )
                for its in range(w // P):
                    pav = psum_av.tile([P, d], f32, tag="pav")
                    prs = psum_rs.tile([P, 1], f32, tag="prs")
                    for jt in range(J):
                        a_t = attnT[:, jt, its * P : (its + 1) * P]
                        nc.tensor.matmul(
                            pav,
                            lhsT=a_t,
                            rhs=v[:, jt, :],
                            start=(jt == 0),
                            stop=(jt == J - 1),
                        )
                        nc.tensor.matmul(
                            prs,
                            lhsT=a_t,
                            rhs=onesc,
                            start=(jt == 0),
                            stop=(jt == J - 1),
                        )
                    rinv = small.tile([P, 1], f32)
                    nc.vector.reciprocal(rinv, prs)
                    outt = work.tile([P, d], f32, tag="outt")
                    nc.vector.scalar_tensor_tensor(
                        out=outt,
                        in0=pav,
                        scalar=rinv,
                        in1=bvb_sb,
                        op0=Mult,
                        op1=Add,
                    )
                    r0 = ibl + its * P
                    nc.sync.dma_start(out=out_d[r0 : r0 + P, :], in_=outt)
                if hook is not None and ibl == 0:
                    hook()
                    hook = None

        def emit_meanv():
            # meanv = mean(x) @ WvT + bv; placed mid-attention so the
            # vector reduces and the 5 matmuls hide behind the S/AV stream.
            xmean_f = small.tile([P, dc_n], f32, name="xmean_f")
            for dc in range(dc_n):
                nc.vector.reduce_sum(
                    xmean_f[:, dc : dc + 1],
                    xm_sb[:, dc, :],
                    axis=mybir.AxisListType.X,
                )
            xmean_b = small.tile([P, dc_n], mm_dt, name="xmean_b")
            nc.vector.tensor_copy(xmean_b, xmean_f)
            pm = psum_m.tile([1, d], f32, tag="pm2", name="pm")
            for dc in range(dc_n):
                nc.tensor.matmul(
                    pm,
                    lhsT=xmean_b[:, dc : dc + 1],
                    rhs=wvT_sb[:, dc, :],
                    start=(dc == 0),
                    stop=(dc == dc_n - 1),
                )
            meanv_sb = small.tile([1, d], f32, name="meanv_sb")
            nc.scalar.activation(meanv_sb, pm, Ident, bias=0.0, scale=1.0 / n)
            meanv2_sb = small.tile([1, d], f32, name="meanv2_sb")
            nc.vector.scalar_tensor_tensor(
                out=meanv2_sb,
                in0=meanv_sb,
                scalar=1.0,
                in1=bvb_sb[0:1, :],
                op0=Mult,
                op1=Add,
            )
            nc.sync.dma_start(out=meanv_d[:, :], in_=meanv2_sb)

        run_bank(banks[0], hook=emit_meanv)
        run_bank(banks[1])

    nc.compile()
    return nc


def make_banked_schedule(lens):
    """Pack (batch, query-tile-start) bins into the template.

    Returns (bank0_bins, bank1_bins, J1) with bins padded to length B
    with None, or None if this length multiset doesn't fit.
    """
    nti = [(int(L) + P - 1) // P for L in lens]
    bank0, bank1 = [], []
    small_ntj = [t for t in nti if t <= J1_MAX]
    J1 = max(small_ntj) if small_ntj else 1
    for b, L in enumerate(lens):
        ntj = nti[b]
        if ntj > J1:
            if ntj > J0:
                return None
            for s in range(0, nti[b], G0):
                bank0.append((b, s))
        else:
            for s in range(0, nti[b], G1):
                bank1.append((b, s))
    if len(bank0) > B or len(bank1) > B:
        return None
    bank0 += [None] * (B - len(bank0))
    bank1 += [None] * (B - len(bank1))
    return bank0, bank1, J1


def make_banked_in_maps(x, lens, wqT, wkT, wvT, bq, bk, bv1, sched):
    bf16 = ml_dtypes.bfloat16
    bank0, bank1, J1 = sched
    xT = [np.ascontiguousarray(x[b].T).astype(bf16) for b in range(B)]
    idx = np.arange(N)

    def maskj_for(L, J):
        m = np.where(idx[: J * P] < L, 0.0, MASK_VAL).astype(np.float32)
        return np.ascontiguousarray(m.reshape(J, P).T)  # [P, J]

    zmask0 = np.zeros((P, J0), np.float32)
    zmask1 = np.zeros((P, J1), np.float32)
    zx = np.zeros((D, 1), bf16)  # broadcast-compatible zero source

    in_maps = []
    for c in range(B):
        im = {
            "wqT": wqT,
            "wkT": wkT,
            "wvT": wvT,
            "bq": bq,
            "bk": bk,
            "bv1": bv1,
            "xm": xT[c],
        }
        for bi, (J, G, bins, zmask) in enumerate(
            ((J0, G0, bank0, zmask0), (J1, G1, bank1, zmask1))
        ):
            bin_ = bins[c]
            if bin_ is None:
                im[f"xb{bi}"] = np.zeros((D, J * P), bf16)
                im[f"xq{bi}"] = np.zeros((D, G * P), bf16)
                im[f"maskj{bi}"] = zmask
            else:
                b, s = bin_
                im[f"xb{bi}"] = np.ascontiguousarray(xT[b][:, : J * P])
                im[f"xq{bi}"] = np.ascontiguousarray(
                    xT[b][:, s * P : (s + G) * P]
                )
                im[f"maskj{bi}"] = maskj_for(int(lens[b]), J)
        in_maps.append(im)
    return in_maps


def assemble_banked(results, lens, sched):
    bank0, bank1, _J1 = sched
    out = np.empty((B, N, D), np.float32)
    meanv = {}
    for c in range(B):
        meanv[c] = np.asarray(results[c]["meanv"], np.float32).reshape(D)
    for b in range(B):
        out[b, :, :] = meanv[b][None, :]
    for bi, (G, bins) in enumerate(((G0, bank0), (G1, bank1))):
        for c in range(B):
            bin_ = bins[c]
            if bin_ is None:
                continue
            b, s = bin_
            L = int(lens[b])
            r0 = s * P
            r1 = min((s + G) * P, L)
            if r1 <= r0:
                continue
            o = np.asarray(results[c][f"out{bi}"], np.float32)
            out[b, r0:r1, :] = o[: r1 - r0, :]
    return out


# --------------------------------------------------------------------------
# dense fallback program (one batch per core)
# --------------------------------------------------------------------------


def build_attention_nc(n=N, d=D, mm_dt=mybir.dt.bfloat16, debug=False):
    """Dense per-batch program (fallback). S-transposed layout."""
    dc_n = d // P
    nt = n // P
    nb = n // FB
    Mult = mybir.AluOpType.mult
    Add = mybir.AluOpType.add

    nc = bacc.Bacc(None, target_bir_lowering=False, debug=debug)

    xT_d = nc.declare_dram_parameter("xT", [d, n], mm_dt, isOutput=False)
    wqT_d = nc.declare_dram_parameter("wqT", [d, d], mm_dt, isOutput=False)
    wkT_d = nc.declare_dram_parameter("wkT", [d, d], mm_dt, isOutput=False)
    wvT_d = nc.declare_dram_parameter("wvT", [d, d], mm_dt, isOutput=False)
    bq_d = nc.declare_dram_parameter("bq", [d], f32, isOutput=False)
    bk_d = nc.declare_dram_parameter("bk", [d], f32, isOutput=False)
    bv_d = nc.declare_dram_parameter("bv1", [1, d], mm_dt, isOutput=False)
    maskj_d = nc.declare_dram_parameter("maskj", [P, nt], f32, isOutput=False)
    out_d = nc.declare_dram_parameter("out", [n, d], f32, isOutput=True)
    meanv_d = nc.declare_dram_parameter("meanv", [1, d], f32, isOutput=True)

    with tile.TileContext(nc) as tc, ExitStack() as ctx:
        const = ctx.enter_context(tc.tile_pool(name="const", bufs=1))
        big = ctx.enter_context(tc.tile_pool(name="big", bufs=1))
        work = ctx.enter_context(tc.tile_pool(name="work", bufs=2))
        small = ctx.enter_context(tc.tile_pool(name="small", bufs=4))
        psum_s = ctx.enter_context(tc.tile_pool(name="psum_s", bufs=2, space="PSUM"))
        psum_av = ctx.enter_context(tc.tile_pool(name="psum_av", bufs=2, space="PSUM"))
        psum_rs = ctx.enter_context(tc.tile_pool(name="psum_rs", bufs=2, space="PSUM"))
        psum_m = ctx.enter_context(tc.tile_pool(name="psum_m", bufs=1, space="PSUM"))

        ones1 = const.tile([1, P], mm_dt)
        nc.vector.memset(ones1, 1.0)
        onesc = const.tile([P, 1], mm_dt)
        nc.vector.memset(onesc, 1.0)
        maskj_sb = const.tile([P, nt], f32)
        nc.sync.dma_start(out=maskj_sb, in_=maskj_d[:, :])
        bv_sb = const.tile([1, d], mm_dt)
        nc.sync.dma_start(out=bv_sb, in_=bv_d[:, :])
        bq_sb = const.tile([P, dc_n], f32)
        nc.sync.dma_start(out=bq_sb, in_=bq_d.ap().rearrange("(c p) -> p c", p=P))
        bk_sb = const.tile([P, dc_n], f32)
        nc.sync.dma_start(out=bk_sb, in_=bk_d.ap().rearrange("(c p) -> p c", p=P))

        xT_sb = big.tile([P, dc_n, n], mm_dt)
        wqT_sb = big.tile([P, dc_n, d], mm_dt)
        wkT_sb = big.tile([P, dc_n, d], mm_dt)
        wvT_sb = big.tile([P, dc_n, d], mm_dt)
        for dc in range(dc_n):
            nc.sync.dma_start(
                out=wqT_sb[:, dc, :], in_=wqT_d[dc * P : (dc + 1) * P, :]
            )
        for dc in range(dc_n):
            nc.sync.dma_start(
                out=xT_sb[:, dc, :FB], in_=xT_d[dc * P : (dc + 1) * P, :FB]
            )
        for dc in range(dc_n):
            nc.sync.dma_start(
                out=wkT_sb[:, dc, :], in_=wkT_d[dc * P : (dc + 1) * P, :]
            )
        for dc in range(dc_n):
            nc.sync.dma_start(
                out=xT_sb[:, dc, FB:], in_=xT_d[dc * P : (dc + 1) * P, FB:]
            )
        for dc in range(dc_n):
            nc.sync.dma_start(
                out=wvT_sb[:, dc, :], in_=wvT_d[dc * P : (dc + 1) * P, :]
            )

        qT_sb = big.tile([P, dc_n, n], mm_dt)
        kT_sb = big.tile([P, dc_n, n], mm_dt)
        v_sb = big.tile([P, nt, d], mm_dt)

        for wT_sb, b_sb, oT_sb in ((wqT_sb, bq_sb, qT_sb), (wkT_sb, bk_sb, kT_sb)):
            for ib in range(nb):
                for ec in range(dc_n):
                    ps = psum_s.tile([P, FB], f32, tag="ps")
                    for dc in range(dc_n):
                        nc.tensor.matmul(
                            ps,
                            lhsT=wT_sb[:, dc, ec * P : (ec + 1) * P],
                            rhs=xT_sb[:, dc, ib * FB : (ib + 1) * FB],
                            start=(dc == 0),
                            stop=(dc == dc_n - 1),
                        )
                    nc.vector.tensor_scalar(
                        out=oT_sb[:, ec, ib * FB : (ib + 1) * FB],
                        in0=ps,
                        scalar1=b_sb[:, ec : ec + 1],
                        scalar2=None,
                        op0=Add,
                    )

        pbv = psum_m.tile([P, d], f32, tag="pm")
        nc.tensor.matmul(pbv, lhsT=ones1, rhs=bv_sb, start=True, stop=True)
        bvb_sb = small.tile([P, d], f32)
        nc.vector.tensor_copy(bvb_sb, pbv)

        for jt in range(nt):
            ps = psum_av.tile([P, d], f32, tag="pav")
            for dc in range(dc_n):
                nc.tensor.matmul(
                    ps,
                    lhsT=xT_sb[:, dc, jt * P : (jt + 1) * P],
                    rhs=wvT_sb[:, dc, :],
                    start=(dc == 0),
                    stop=(dc == dc_n - 1),
                )
            nc.vector.scalar_tensor_tensor(
                out=v_sb[:, jt, :], in0=ps, scalar=1.0, in1=bvb_sb, op0=Mult, op1=Add
            )

        xmean_f = small.tile([P, dc_n], f32)
        for dc in range(dc_n):
            nc.vector.reduce_sum(
                xmean_f[:, dc : dc + 1], xT_sb[:, dc, :], axis=mybir.AxisListType.X
            )
        xmean_b = small.tile([P, dc_n], mm_dt)
        nc.vector.tensor_copy(xmean_b, xmean_f)
        pm = psum_m.tile([1, d], f32, tag="pm2")
        for dc in range(dc_n):
            nc.tensor.matmul(
                pm,
                lhsT=xmean_b[:, dc : dc + 1],
                rhs=wvT_sb[:, dc, :],
                start=(dc == 0),
                stop=(dc == dc_n - 1),
            )
        meanv_sb = small.tile([1, d], f32)
        nc.scalar.activation(meanv_sb, pm, Ident, bias=0.0, scale=1.0 / n)
        meanv2_sb = small.tile([1, d], f32)
        nc.vector.scalar_tensor_tensor(
            out=meanv2_sb,
            in0=meanv_sb,
            scalar=1.0,
            in1=bvb_sb[0:1, :],
            op0=Mult,
            op1=Add,
        )
        nc.sync.dma_start(out=meanv_d[:, :], in_=meanv2_sb)

        for ibl in range(nb):
            attnT = work.tile([P, nt, FB], mm_dt)
            for jt in range(nt):
                ps = psum_s.tile([P, FB], f32, tag="ps")
                for dc in range(dc_n):
                    nc.tensor.matmul(
                        ps,
                        lhsT=kT_sb[:, dc, jt * P : (jt + 1) * P],
                        rhs=qT_sb[:, dc, ibl * FB : (ibl + 1) * FB],
                        start=(dc == 0),
                        stop=(dc == dc_n - 1),
                    )
                nc.scalar.activation(
                    attnT[:, jt, :],
                    ps,
                    Exp,
                    bias=maskj_sb[:, jt : jt + 1],
                    scale=SCALE,
                )
            for its in range(nb):
                it = ibl * nb + its
                pav = psum_av.tile([P, d], f32, tag="pav")
                prs = psum_rs.tile([P, 1], f32, tag="prs")
                for jt in range(nt):
                    a_t = attnT[:, jt, its * P : (its + 1) * P]
                    nc.tensor.matmul(
                        pav,
                        lhsT=a_t,
                        rhs=v_sb[:, jt, :],
                        start=(jt == 0),
                        stop=(jt == nt - 1),
                    )
                    nc.tensor.matmul(
                        prs,
                        lhsT=a_t,
                        rhs=onesc,
                        start=(jt == 0),
                        stop=(jt == nt - 1),
                    )
                rinv = small.tile([P, 1], f32)
                nc.vector.reciprocal(rinv, prs)
                outt = work.tile([P, d], f32, tag="outt")
                nc.vector.tensor_scalar(
                    out=outt, in0=pav, scalar1=rinv, scalar2=None, op0=Mult
                )
                nc.sync.dma_start(out=out_d[it * P : (it + 1) * P, :], in_=outt)

    nc.compile()
    return nc


def _marshal_weights(Wq, bq, Wk, bk, Wv, bv):
    bf16 = ml_dtypes.bfloat16
    return (
        np.ascontiguousarray(np.asarray(Wq, np.float32).T).astype(bf16),
        np.ascontiguousarray(np.asarray(Wk, np.float32).T).astype(bf16),
        np.ascontiguousarray(np.asarray(Wv, np.float32).T).astype(bf16),
        np.asarray(bq, np.float32),
        np.asarray(bk, np.float32),
        np.asarray(bv, np.float32).reshape(1, D).astype(bf16),
    )


def make_in_maps(x, event_lengths, Wq, bq, Wk, bk, Wv, bv, n=N, d=D):
    """Dense fallback marshaling: one batch element per core."""
    bf16 = ml_dtypes.bfloat16
    x = np.asarray(x, dtype=np.float32)
    lens = np.asarray(event_lengths).astype(np.int64)
    wqT, wkT, wvT, bq, bk, bv1 = _marshal_weights(Wq, bq, Wk, bk, Wv, bv)
    idx = np.arange(n)
    in_maps = []
    for b in range(x.shape[0]):
        L = int(lens[b])
        maskj = np.where(idx < L, 0.0, MASK_VAL).astype(np.float32)
        maskj = np.ascontiguousarray(maskj.reshape(n // P, P).T)
        in_maps.append(
            {
                "xT": np.ascontiguousarray(x[b].T).astype(bf16),
                "wqT": wqT,
                "wkT": wkT,
                "wvT": wvT,
                "bq": bq,
                "bk": bk,
                "bv1": bv1,
                "maskj": maskj,
            }
        )
    return in_maps


def assemble_output(results, event_lengths, n=N, d=D):
    """Dense fallback: stack per-core outputs; paste meanv into padded rows."""
    lens = np.asarray(event_lengths).astype(np.int64)
    outs = []
    for b, r in enumerate(results):
        o = np.array(r["out"], np.float32)
        L = int(lens[b])
        if L < n:
            o[L:, :] = np.asarray(r["meanv"], np.float32).reshape(1, d)
        outs.append(o)
    return np.stack(outs, axis=0)


_NC_CACHE = {}


def prepare(x, event_lengths, Wq, bq, Wk, bk, Wv, bv):
    """Returns (nc, in_maps, assemble_fn)."""
    x = np.asarray(x, dtype=np.float32)
    lens = np.asarray(event_lengths).astype(np.int64)
    sched = make_banked_schedule(lens)
    if sched is not None:
        key = ("banked", sched[2])
        if key not in _NC_CACHE:
            _NC_CACHE[key] = build_banked_nc(sched[2])
        wqT, wkT, wvT, bqf, bkf, bv1 = _marshal_weights(Wq, bq, Wk, bk, Wv, bv)
        in_maps = make_banked_in_maps(x, lens, wqT, wkT, wvT, bqf, bkf, bv1, sched)
        return (
            _NC_CACHE[key],
            in_maps,
            lambda results: assemble_banked(results, lens, sched),
        )
    if "dense" not in _NC_CACHE:
        _NC_CACHE["dense"] = build_attention_nc()
    in_maps = make_in_maps(x, event_lengths, Wq, bq, Wk, bk, Wv, bv)
    return (
        _NC_CACHE["dense"],
        in_maps,
        lambda results: assemble_output(results, event_lengths),
    )


def kernel(x, event_lengths, Wq, bq, Wk, bk, Wv, bv):
    from concourse.bass_utils import run_bass_kernel_spmd

    nc, in_maps, assemble = prepare(x, event_lengths, Wq, bq, Wk, bk, Wv, bv)
    res = run_bass_kernel_spmd(nc, in_maps, core_ids=list(range(B)))
    return assemble(res.results)
